# revision 21
# baseline (speedup 1.0000x reference)
"""Trainium2 Bass kernel for nn_Nequix (e3nn-style message-passing layer).

Sharding: nodes are greedily packed into 32 bins (8 cores x 4 windows of 128
receiver slots) balancing edge counts, so every window pads to EW=4096 edges.
Node features and weights replicated; no collectives.

Per-core pipeline (all windows uniform):
  phaseY   y = x @ W1 for all 4096 node slots, quarter-pipelined x loads,
           y rows written to DRAM (y_d) for gathering
  radial   2-slot packed radial MLP -> h3 slabs per window
  gather   m = y[senders] via SWDGE dma_gather, one call per 8-tile chunk
  L3       per-tile h3 lhsT -> w [128e, 4C] PSUM, ACT evac
  products 9 DVE slab mults m (.) w per chunk (+1 ACT copy)
  scatter  one-hot matmuls (Y1 rides in the one-hot weights), pipelined one
           chunk behind products; agg PSUM banks A=[s0|v1x|v1y|v1z]
           B=[v0x|s1x|s1y|v0y] C=[s1z|v0z]
  final    s1 presummed on DVE, 8 transposes, linear_2 + species skip with
           384-col folded matmuls, silu gates; emitted inside the next window
"""
import math
import os
import numpy as np

N, E, C, NS, RB, H = 4000, 128000, 128, 5, 8, 64
AVG_N = 32.0
NCORES = 8
WIN = 128
NWIN = 4
NBIN = NCORES * NWIN
SQ3 = math.sqrt(3.0)
CH = 8                                   # edge tiles per compute chunk


def _prep_host(inputs):
    import ml_dtypes
    bf = ml_dtypes.bfloat16
    f32 = np.float32

    xs = np.asarray(inputs["x_scalars"], f32)
    xv = np.asarray(inputs["x_vectors"], f32)
    ev = np.asarray(inputs["edge_vectors"], f32)
    rb = np.asarray(inputs["radial_basis"], f32)
    W1_0 = np.asarray(inputs["W1_0"], f32)
    W1_1 = np.asarray(inputs["W1_1"], f32)
    w0 = np.asarray(inputs["rmlp_w0"], f32)
    w1 = np.asarray(inputs["rmlp_w1"], f32)
    w2 = np.asarray(inputs["rmlp_w2"], f32)
    w3 = np.asarray(inputs["rmlp_w3"], f32).copy()
    W2_0 = np.asarray(inputs["W2_0"], f32)
    W2_1 = np.asarray(inputs["W2_1"], f32)
    Wsk0 = np.asarray(inputs["Wsk0"], f32)
    Wsk1 = np.asarray(inputs["Wsk1"], f32)
    species = np.asarray(inputs["species"]).astype(np.int64)
    send = np.asarray(inputs["senders"]).astype(np.int64)
    recv = np.asarray(inputs["receivers"]).astype(np.int64)

    inv_c = f32(1.0 / math.sqrt(C))
    W1_0f = W1_0 * inv_c
    W1_1f = W1_1 * inv_c
    w3f = w3 * f32(1.0 / math.sqrt(AVG_N))
    w3f[:, C:2 * C] *= f32(1.0 / SQ3)
    inv_2c = f32(1.0 / math.sqrt(2 * C))
    W2_0f = W2_0 * inv_2c
    W2_1f = W2_1 * inv_2c
    Wsk0f = Wsk0 * inv_c          # [NS, C, 2C]
    Wsk1f = Wsk1 * inv_c          # [NS, C, C]

    # edge geometry (host): Y1 = sqrt(3) * unit(edge_vectors)
    r = np.sqrt((ev * ev).sum(1, keepdims=True))
    Y1 = SQ3 * ev / np.maximum(r, 1e-12)                               # [E,3]

    # ---- balanced node->bin packing (LPT on degree, cap 128 nodes/bin)
    deg = np.bincount(recv, minlength=N)
    order = np.argsort(-deg, kind="stable")
    bin_edges = np.zeros(NBIN, np.int64)
    bin_count = np.zeros(NBIN, np.int64)
    bin_nodes = [[] for _ in range(NBIN)]
    for n in order:
        cand = np.nonzero(bin_count < WIN)[0]
        b = cand[np.argmin(bin_edges[cand])]
        bin_edges[b] += deg[n]
        bin_count[b] += 1
        bin_nodes[b].append(n)
    node_bin = np.zeros(N, np.int64)
    node_slot = np.zeros(N, np.int64)
    for b in range(NBIN):
        for j, n in enumerate(bin_nodes[b]):
            node_bin[n] = b
            node_slot[n] = j

    EW = int(((bin_edges.max() + 255) // 256) * 256)
    TW = EW // 128
    EPAD = EW * NWIN
    TT = EPAD // 128

    ebin = node_bin[recv]
    rloc_all = node_slot[recv]

    # ---- shared constants
    w0b = np.zeros((16, 128), f32)
    w0b[:RB, :H] = w0
    w0b[RB:2 * RB, H:2 * H] = w0
    w1b = np.zeros((128, 128), f32)
    w1b[:H, :H] = w1
    w1b[H:, H:] = w1
    w2b = np.zeros((128, 128), f32)
    w2b[:H, :H] = w2
    w2b[H:, H:] = w2
    w3d = np.concatenate([w3f, w3f], axis=0)                           # [128,4C]

    W20L = np.stack([W2_0f[0:128, :], W2_0f[128:256, :]], axis=1)      # [128,2,256]
    W21L = np.stack([W2_1f[0:128, :], W2_1f[128:256, :]], axis=1)      # [128,2,128]
    Wsk0L = Wsk0f.transpose(1, 0, 2)                                   # [128,NS,256]
    Wsk1L = Wsk1f.transpose(1, 0, 2)                                   # [128,NS,128]

    NPAD = 4096
    xall = np.zeros((C, 4, NPAD), f32)
    xall[:, 0, :N] = xs.T
    for i in range(3):
        xall[:, 1 + i, :N] = xv[:, :, i].T

    consts = dict(
        xall=xall.astype(bf),
        W10=W1_0f.astype(bf), W11=W1_1f.astype(bf),
        w0b=w0b.astype(bf), w1b=w1b.astype(bf), w2b=w2b.astype(bf),
        w3d=w3d.astype(bf),
        W20=W20L.astype(bf), W21=W21L.astype(bf),
        Wsk0=Wsk0L.astype(bf), Wsk1=Wsk1L.astype(bf),
    )

    # ---- per-core tensors
    cores = []
    node_order = np.full((NCORES, NWIN, WIN), -1, np.int64)
    for i in range(NCORES):
        send_p = np.zeros(EPAD, np.int64)
        rloc_p = np.zeros(EPAD, np.int64)
        val_p = np.zeros(EPAD, f32)
        rb_p = np.zeros((EPAD, RB), f32)
        Y1_p = np.zeros((EPAD, 3), f32)
        for w in range(NWIN):
            b = i * NWIN + w
            pw = np.nonzero(ebin == b)[0]
            pw = pw[np.argsort(send[pw], kind="stable")]
            k = len(pw)
            sl = slice(w * EW, w * EW + k)
            send_p[sl] = send[pw]
            rloc_p[sl] = rloc_all[pw]
            val_p[sl] = 1.0
            rb_p[sl] = rb[pw]
            Y1_p[sl] = Y1[pw]
            node_order[i, w, :len(bin_nodes[b])] = bin_nodes[b]

        # gather indices, int16 (pad slots gather row 0; one-hot row is zero)
        # chunk k of each window only gathers y rows < 1024*(k+2) (senders
        # sorted per window; asserted here, baked into the program's deps)
        for w in range(NWIN):
            for kc in range(2):
                seg = send_p[w * EW + kc * 1024: w * EW + (kc + 1) * 1024]
                assert seg.max(initial=0) < 1024 * (kc + 2), "sender skew"
        idx16 = send_p.astype(np.int16).reshape(EPAD // 16, 16).T
        sendidx = np.tile(idx16, (8, 1))                               # [128, EPAD//16]

        # OHW[p, t, 0, n] = onehot; [p, t, 1+i, n] = onehot * Y1_i
        ohw = np.zeros((EPAD, 4, WIN), f32)
        ar = np.arange(EPAD)
        ohw[ar, 0, rloc_p] = val_p
        for j in range(3):
            ohw[ar, 1 + j, rloc_p] = val_p * Y1_p[:, j]
        OHW = ohw.reshape(TT, 128, 4, WIN).transpose(1, 0, 2, 3)

        # rb2[s*8+r, w*(EW/2) + P*128 + p] = rb[edge (w, (2P+s)*128+p), r]
        arr = rb_p.reshape(NWIN, TW // 2, 2, 128, RB)
        rb2 = arr.transpose(2, 4, 0, 1, 3).reshape(16, EPAD // 2)

        xs_my = np.zeros((NWIN * WIN, C), f32)
        xv_my = np.zeros((NWIN * WIN, C, 3), f32)
        soh = np.zeros((NWIN * WIN, NS), f32)
        for w in range(NWIN):
            b = i * NWIN + w
            nb = bin_nodes[b]
            xs_my[w * WIN:w * WIN + len(nb)] = xs[nb]
            xv_my[w * WIN:w * WIN + len(nb)] = xv[nb]
            soh[np.arange(w * WIN, w * WIN + len(nb)), species[nb]] = 1.0
        xskT = np.einsum("nc,nk->ckn", xs_my, soh)                     # [128,NS,512]
        # [C, NS, NWIN, 3, WIN] so the (k, w) slice is 384 contiguous cols
        xvkT = np.einsum("nci,nk->ckin", xv_my, soh).reshape(
            C, NS, 3, NWIN, WIN).transpose(0, 1, 3, 2, 4).reshape(
            C, NS, NWIN, 3 * WIN)

        cores.append(dict(
            sendidx=np.ascontiguousarray(sendidx),
            OHW=np.ascontiguousarray(OHW.astype(bf)),
            rb2=np.ascontiguousarray(rb2.astype(bf)),
            xskT=np.ascontiguousarray(xskT.astype(bf)),
            xvkT=np.ascontiguousarray(xvkT.astype(bf)),
        ))
    return consts, cores, EW, node_order


def _build_program(EW):
    import concourse.bacc as bacc
    import concourse.mybir as mybir
    import concourse.tile as tile
    from concourse.masks import make_identity

    f32 = mybir.dt.float32
    bf = mybir.dt.bfloat16
    AF = mybir.ActivationFunctionType
    OP = mybir.AluOpType

    TW = EW // 128
    EPAD = EW * NWIN
    TT = EPAD // 128

    nc = bacc.Bacc("TRN2", target_bir_lowering=False)

    def param(name, shape, dtype):
        return nc.declare_dram_parameter(name, list(shape), dtype, isOutput=False)

    NPAD = 4096
    i16 = mybir.dt.int16
    xall_d = param("xall", (C, 4, NPAD), bf)
    sendidx_d = param("sendidx", (128, EPAD // 16), i16)
    W10_d = param("W10", (C, C), bf)
    W11_d = param("W11", (C, C), bf)
    w0b_d = param("w0b", (16, 128), bf)
    w1b_d = param("w1b", (128, 128), bf)
    w2b_d = param("w2b", (128, 128), bf)
    w3d_d = param("w3d", (128, 4 * C), bf)
    W20_d = param("W20", (C, 2, 2 * C), bf)
    W21_d = param("W21", (C, 2, C), bf)
    Wsk0_d = param("Wsk0", (C, NS, 2 * C), bf)
    Wsk1_d = param("Wsk1", (C, NS, C), bf)
    OHW_d = param("OHW", (128, TT, 4, WIN), bf)
    rb2_d = param("rb2", (16, EPAD // 2), bf)
    xskT_d = param("xskT", (C, NS, NWIN * WIN), bf)
    xvkT_d = param("xvkT", (C, NS, NWIN, 3 * WIN), bf)
    outT_d = nc.declare_dram_parameter("outT", [4 * C, NWIN * WIN], f32,
                                       isOutput=True)

    with tile.TileContext(nc) as tc:
        with (
            tc.tile_pool(name="dram", bufs=1, space="DRAM") as dpool,
            tc.tile_pool(name="const", bufs=1) as cpool,
            tc.tile_pool(name="xload", bufs=2) as xpool,
            tc.tile_pool(name="rbload", bufs=2) as rbpool,
            tc.tile_pool(name="hslab", bufs=2) as hpool,
            tc.tile_pool(name="eload", bufs=2) as epool,
            tc.tile_pool(name="mw", bufs=3) as mwpool,
            tc.tile_pool(name="prod", bufs=2) as ppool,
            tc.tile_pool(name="fin", bufs=2) as fpool,
            tc.tile_pool(name="ps_w", bufs=4, space="PSUM") as ps_w,
            tc.tile_pool(name="ps_f", bufs=1, space="PSUM") as ps_f,
            tc.tile_pool(name="ps_agg", bufs=1, space="PSUM") as ps_agg,
        ):
            y_d = dpool.tile([NPAD, 4 * C], bf)

            def cload(dram, shape, dtype):
                t = cpool.tile(list(shape), dtype, tag=dram.name)
                nc.scalar.dma_start(t[:], dram[:])
                return t

            W10_s = cload(W10_d, (C, C), bf)
            W11_s = cload(W11_d, (C, C), bf)
            w0b_s = cload(w0b_d, (16, 128), bf)
            w1b_s = cload(w1b_d, (128, 128), bf)
            w2b_s = cload(w2b_d, (128, 128), bf)
            w3d_s = cload(w3d_d, (128, 4 * C), bf)
            sendidx_s = cload(sendidx_d, (128, EPAD // 16), i16)

            # ====== phase Y: y = x @ W1 (all nodes), quarter-pipelined ======
            # y_d written partition-major ([128, 32, 512]); gather indices are
            # host-remapped to match, so each quarter is a single DMA
            y_v = y_d[:].rearrange("(c p) f -> p c f", p=128)
            aggY = ps_agg.tile([128, 3, 512], f32, tag="agg")
            yq = None
            for g in range(8):
                xe = xpool.tile([C, 4, 512], bf, tag="xq", bufs=3)
                nc.sync.dma_start(xe[:], xall_d[:, :, g * 512:(g + 1) * 512])
                if g % 2 == 0:
                    yq = xpool.tile([128, 8, 4 * C], bf, tag="yq", name="yq")
                for nch in range(4):
                    k = g * 4 + nch
                    csl = slice(nch * 128, (nch + 1) * 128)
                    k7 = k % 7
                    if k7 < 4:
                        psy = ps_w.tile([128, 4 * C], f32, tag="w", name="psy")
                    else:
                        psy = aggY[:, k7 - 4, :]
                    nc.tensor.matmul(psy[:, 0:C], lhsT=xe[:, 0, csl],
                                     rhs=W10_s[:], start=True, stop=True)
                    for i in range(3):
                        nc.tensor.matmul(psy[:, (1 + i) * C:(2 + i) * C],
                                         lhsT=xe[:, 1 + i, csl],
                                         rhs=W11_s[:], start=True, stop=True)
                    if k % 2 == 0:
                        nc.vector.tensor_copy(yq[:, k % 8, :], psy[:])
                    else:
                        nc.scalar.copy(yq[:, k % 8, :], psy[:])
                if g % 2 == 1:
                    q = g // 2
                    nc.scalar.dma_start(y_v[:, q * 8:(q + 1) * 8, :], yq[:])

            ncol0 = EW // 2
            rb2_w0 = rbpool.tile([16, ncol0], bf, tag="rb2")
            nc.scalar.dma_start(rb2_w0[:], rb2_d[:, 0:ncol0])
            xskT_w0 = fpool.tile([C, NS, WIN], bf, tag="xsk")
            nc.scalar.dma_start(xskT_w0[:], xskT_d[:, :, 0:WIN])
            xvkT_w0 = fpool.tile([C, NS, 3 * WIN], bf, tag="xvk")
            nc.scalar.dma_start(xvkT_w0[:], xvkT_d[:, :, 0, :])

            # final-stage constants (not needed until window 1)
            W20_s = cload(W20_d, (C, 2, 2 * C), bf)
            W21_s = cload(W21_d, (C, 2, C), bf)
            Wsk0_s = cload(Wsk0_d, (C, NS, 2 * C), bf)
            Wsk1_s = cload(Wsk1_d, (C, NS, C), bf)
            ident_s = cpool.tile([128, 128], bf)
            make_identity(nc, ident_s[:])

            pending_final = [None]

            for w in range(NWIN):
                # ---------------- radial MLP (2-slot packed) ----------------
                ncol = EW // 2
                if w == 0:
                    rb2_t, xskT_w, xvkT_w = rb2_w0, xskT_w0, xvkT_w0
                else:
                    rb2_t = rbpool.tile([16, ncol], bf, tag="rb2")
                    nc.sync.dma_start(rb2_t[:], rb2_d[:, w * ncol:(w + 1) * ncol])
                    xskT_w = fpool.tile([C, NS, WIN], bf, tag="xsk")
                    nc.sync.dma_start(xskT_w[:],
                                      xskT_d[:, :, w * WIN:(w + 1) * WIN])
                    xvkT_w = fpool.tile([C, NS, 3 * WIN], bf, tag="xvk")
                    nc.sync.dma_start(xvkT_w[:], xvkT_d[:, :, w, :])
                h3 = hpool.tile([128, ncol], bf, tag="h3")
                h1 = hpool.tile([128, 2, 512], bf, tag="h1")
                h2 = hpool.tile([128, 2, 512], bf, tag="h2")

                def radial_pair(cA, cB):
                    # two 512-col chunks interleaved so PE never waits on silu
                    cols = [(cA, 0), (cB, 1)]
                    W = [w0b_s, w1b_s, w2b_s]
                    src_h = [None, h1, h2]
                    ps = {}
                    for l in range(3):
                        for (c0, p) in cols:
                            pst = ps_w.tile([128, 512], f32, tag="w",
                                            name=f"psr{l}{p}")
                            if l == 0:
                                nc.tensor.matmul(pst[:], lhsT=W[0][:],
                                                 rhs=rb2_t[:, c0:c0 + 512],
                                                 start=True, stop=True)
                            else:
                                nc.tensor.matmul(pst[:], lhsT=W[l][:],
                                                 rhs=src_h[l][:, p, :],
                                                 start=True, stop=True)
                            ps[p] = pst
                        for (c0, p) in cols:
                            if l < 2:
                                nc.scalar.activation(src_h[l + 1][:, p, :],
                                                     ps[p][:], AF.Silu)
                            else:
                                nc.scalar.activation(h3[:, c0:c0 + 512],
                                                     ps[p][:], AF.Silu)

                # banks: A=[s0|v1x|v1y|v1z]  B=[v0x|s1x|s1y|v0y]  Cb=[s1z|v0z]
                agg = ps_agg.tile([128, 3, 512], f32, tag="agg")

                def emit_scatter(st):
                    (s_t0, s_t1, s_ohw, s_P1, s_P2, s_P3) = st
                    for tl in range(s_t1 - s_t0):
                        t = s_t0 + tl
                        first = (t == 0)
                        last = (t == TW - 1)
                        oh = s_ohw[:, tl, 0, :]
                        ohx = s_ohw[:, tl, 1, :]
                        ohy = s_ohw[:, tl, 2, :]
                        ohz = s_ohw[:, tl, 3, :]
                        nc.tensor.matmul(agg[:, 0, :], lhsT=oh,
                                         rhs=s_P1[:, tl, 0:512],
                                         start=first, stop=last,
                                         skip_group_check=True)
                        nc.tensor.matmul(agg[:, 1, 0:256], lhsT=ohx,
                                         rhs=s_P2[:, tl, 128:384],
                                         start=first, stop=False,
                                         skip_group_check=True)
                        nc.tensor.matmul(agg[:, 1, 256:512], lhsT=ohy,
                                         rhs=s_P2[:, tl, 0:256],
                                         start=False, stop=last,
                                         skip_group_check=True)
                        nc.tensor.matmul(agg[:, 2, 0:256], lhsT=ohz,
                                         rhs=s_P3[:, tl, 0:256],
                                         start=first, stop=last,
                                         skip_group_check=True)

                pending = None
                chunks = [(t0, min(t0 + CH, TW)) for t0 in range(0, TW, CH)]
                for (t0, t1) in chunks:
                    nt = t1 - t0
                    g0 = w * TW + t0
                    # dense scatter block first (keeps the PE p-state ramped)
                    if pending is not None:
                        emit_scatter(pending)
                        pending = None
                    # radial cols [t0*64, t1*64) feed edge tiles [t0, t1);
                    # pairs emitted at chunks 0 and 2
                    if t0 % (2 * CH) == 0:
                        radial_pair(t0 * 64, (t0 + CH) * 64)
                    if t0 == 0 and pending_final[0] is not None:
                        pending_final[0]()
                        pending_final[0] = None
                    # ---- loads for this chunk
                    ohw_t = epool.tile([128, CH, 4, WIN], bf, tag="ohw", bufs=2)
                    nc.sync.dma_start(ohw_t[:, :nt], OHW_d[:, g0:g0 + nt, :, :])

                    # ---- m = y[send] via SWDGE gather
                    m_sb = mwpool.tile([128, CH, 512], bf, tag="m", bufs=4)
                    w_sb = mwpool.tile([128, CH, 512], bf, tag="w", bufs=3)
                    nidx = nt * 128
                    ylim = min(1024 * (t0 // CH + 2), NPAD)
                    with tc.high_priority():
                        nc.gpsimd.dma_gather(
                            m_sb[:, 0:nt, :], y_d[0:ylim, :],
                            sendidx_s[:, g0 * 8:g0 * 8 + nidx // 16],
                            nidx, nidx, 4 * C,
                        )
                    for tl in range(nt):
                        t = t0 + tl
                        s = t % 2
                        P = t // 2
                        psw = ps_w.tile([128, 512], f32, tag="w", name="psw")
                        nc.tensor.matmul(
                            psw[:], lhsT=h3[s * 64:(s + 1) * 64,
                                            P * 128:(P + 1) * 128],
                            rhs=w3d_s[s * 64:(s + 1) * 64, :],
                            start=True, stop=True)
                        if tl % 2 == 0:
                            nc.scalar.copy(w_sb[:, tl, :], psw[:])
                        else:
                            nc.vector.tensor_copy(w_sb[:, tl, :], psw[:])

                    # ---- products (DVE slabs over the chunk)
                    # w_sb cols: [ws0 | ws1' | wv0 | wv1]; m_sb: [m0|m1x|m1y|m1z]
                    P1 = ppool.tile([128, CH, 512], bf, tag="P1", bufs=2)
                    P2 = ppool.tile([128, CH, 384], bf, tag="P2", bufs=2)
                    P3 = ppool.tile([128, CH, 256], bf, tag="P3", bufs=2)

                    def mslice(j):
                        return m_sb[:, 0:nt, j * 128:(j + 1) * 128]

                    def wslice(j):
                        return w_sb[:, 0:nt, j * 128:(j + 1) * 128]

                    # P1 = [m0*ws0 | m1x*wv1 | m1y*wv1 | m1z*wv1]
                    nc.vector.tensor_tensor(out=P1[:, 0:nt, 0:128],
                                            in0=mslice(0), in1=wslice(0), op=OP.mult)
                    for j in range(3):
                        nc.vector.tensor_tensor(
                            out=P1[:, 0:nt, (1 + j) * 128:(2 + j) * 128],
                            in0=mslice(1 + j), in1=wslice(3), op=OP.mult)
                    # P2 = [m1y*ws1' | m0*wv0 | m1x*ws1']
                    nc.vector.tensor_tensor(out=P2[:, 0:nt, 0:128],
                                            in0=mslice(2), in1=wslice(1), op=OP.mult)
                    nc.vector.tensor_tensor(out=P2[:, 0:nt, 128:256],
                                            in0=mslice(0), in1=wslice(2), op=OP.mult)
                    nc.vector.tensor_tensor(out=P2[:, 0:nt, 256:384],
                                            in0=mslice(1), in1=wslice(1), op=OP.mult)
                    # P3 = [m1z*ws1' | m0*wv0 (copy)]
                    nc.vector.tensor_tensor(out=P3[:, 0:nt, 0:128],
                                            in0=mslice(3), in1=wslice(1), op=OP.mult)
                    nc.scalar.copy(P3[:, 0:nt, 128:256], P2[:, 0:nt, 128:256])

                    # ---- scatter pipelined one chunk behind
                    pending = (t0, t1, ohw_t, P1, P2, P3)

                if pending is not None:
                    emit_scatter(pending)
                    pending = None

                # ================= final per window =================
                def make_final(w, agg, xskT_w=xskT_w, xvkT_w=xvkT_w):
                    def emit_final():
                        # agg_sb blocks: 0=s0 1=s1sum 2=v0x 3=v0y 4=v0z
                        #                5=v1x 6=v1y 7=v1z
                        agg_sb = fpool.tile([128, 8, 128], bf, tag="aggsb")
                        nc.scalar.copy(agg_sb[:, 0, :], agg[:, 0, 0:128])
                        nc.scalar.copy(agg_sb[:, 5:8, :], agg[:, 0, 128:512])
                        nc.scalar.copy(agg_sb[:, 2, :], agg[:, 1, 0:128])
                        nc.scalar.copy(agg_sb[:, 3, :], agg[:, 1, 384:512])
                        nc.scalar.copy(agg_sb[:, 4, :], agg[:, 2, 128:256])
                        s1t = fpool.tile([128, 2, 128], f32, tag="s1t")
                        nc.scalar.copy(s1t[:, 0, :], agg[:, 1, 128:256])
                        nc.vector.tensor_tensor(out=s1t[:, 1, :],
                                                in0=agg[:, 1, 256:384],
                                                in1=s1t[:, 0, :], op=OP.add)
                        nc.vector.tensor_tensor(out=agg_sb[:, 1, :],
                                                in0=agg[:, 2, 0:128],
                                                in1=s1t[:, 1, :], op=OP.add)

                        aggT = fpool.tile([128, 8, 128], bf, tag="aggT")
                        for b in range(8):
                            pst = ps_f.tile([128, 128], bf, tag="f", name="pst")
                            nc.tensor.transpose(pst[:], agg_sb[:, b, :],
                                                identity=ident_s[:])
                            if b % 2 == 0:
                                nc.vector.tensor_copy(aggT[:, b, :], pst[:])
                            else:
                                nc.scalar.copy(aggT[:, b, :], pst[:])

                        pss = ps_f.tile([128, 2, 128], f32, tag="f", name="pss")
                        for mch in range(2):
                            nc.tensor.matmul(
                                pss[:, mch, :],
                                lhsT=W20_s[:, 0, mch * 128:(mch + 1) * 128],
                                rhs=aggT[:, 0, :], start=True, stop=False,
                                skip_group_check=True)
                            nc.tensor.matmul(
                                pss[:, mch, :],
                                lhsT=W20_s[:, 1, mch * 128:(mch + 1) * 128],
                                rhs=aggT[:, 1, :], start=False, stop=False,
                                skip_group_check=True)
                            for k in range(NS):
                                nc.tensor.matmul(
                                    pss[:, mch, :],
                                    lhsT=Wsk0_s[:, k, mch * 128:(mch + 1) * 128],
                                    rhs=xskT_w[:, k, :],
                                    start=False, stop=(k == NS - 1),
                                    skip_group_check=True)
                        outs = fpool.tile([128, 128], bf, tag="outs")
                        nc.scalar.activation(outs[:], pss[:, 0, :], AF.Silu)
                        gates = fpool.tile([128, 128], bf, tag="gates")
                        nc.scalar.activation(gates[:], pss[:, 1, :], AF.Silu)

                        psv = ps_f.tile([128, 3, 128], f32, tag="f", name="psv")
                        nc.tensor.matmul(psv[:, :, :], lhsT=W21_s[:, 0, :],
                                         rhs=aggT[:, 2:5, :],
                                         start=True, stop=False,
                                         skip_group_check=True)
                        nc.tensor.matmul(psv[:, :, :], lhsT=W21_s[:, 1, :],
                                         rhs=aggT[:, 5:8, :],
                                         start=False, stop=False,
                                         skip_group_check=True)
                        for k in range(NS):
                            nc.tensor.matmul(
                                psv[:, :, :], lhsT=Wsk1_s[:, k, :],
                                rhs=xvkT_w[:, k, :],
                                start=False, stop=(k == NS - 1),
                                skip_group_check=True)

                        ow = fpool.tile([128, 4, 128], f32, tag="ow")
                        nc.vector.tensor_copy(ow[:, 0, :], outs[:])
                        for i in range(3):
                            nc.vector.tensor_tensor(out=ow[:, 1 + i, :],
                                                    in0=psv[:, i, :],
                                                    in1=gates[:], op=OP.mult)
                        for fch in range(4):
                            nc.sync.dma_start(
                                outT_d[fch * 128:(fch + 1) * 128,
                                       w * 128:(w + 1) * 128],
                                ow[:, fch, :])
                    return emit_final

                pending_final[0] = make_final(w, agg)

            if pending_final[0] is not None:
                pending_final[0]()
                pending_final[0] = None
    nc.compile()
    return nc


_CACHE = {}


def kernel(**inputs):
    from concourse.bass_utils import run_bass_kernel_spmd
    consts, cores, EW, node_order = _prep_host(inputs)
    if EW not in _CACHE:
        _CACHE[EW] = _build_program(EW)
    nc = _CACHE[EW]
    in_maps = []
    for i in range(NCORES):
        m = dict(consts)
        m.update(cores[i])
        in_maps.append(m)
    res = run_bass_kernel_spmd(nc, in_maps, list(range(NCORES)))
    out = np.zeros((N, 4 * C), np.float32)
    for i in range(NCORES):
        outT = np.asarray(res.results[i]["outT"], np.float32)   # [512, 512]
        full = outT.T                                           # [512, 512]
        for w in range(NWIN):
            nodes = node_order[i, w]
            valid = nodes >= 0
            rows = full[w * WIN:(w + 1) * WIN][valid]
            out_s = rows[:, 0:C]
            v = np.stack([rows[:, C:2 * C], rows[:, 2 * C:3 * C],
                          rows[:, 3 * C:]], axis=2).reshape(-1, 3 * C)
            out[nodes[valid]] = np.concatenate([out_s, v], axis=1)
    return out


# revision 23
# speedup vs baseline: 1.0304x; 1.0304x over previous
"""Trainium2 Bass kernel for nn_Nequix (e3nn-style message-passing layer).

Sharding: nodes are greedily packed into 32 bins (8 cores x 4 windows of 128
receiver slots) balancing edge counts, so every window pads to EW=4096 edges.
Node features and weights replicated; no collectives.

Per-core pipeline (all windows uniform):
  phaseY   y = x @ W1 for all 4096 node slots, quarter-pipelined x loads,
           y rows written to DRAM (y_d) for gathering
  radial   2-slot packed radial MLP -> h3 slabs per window
  gather   m = y[senders] via SWDGE dma_gather, one call per 8-tile chunk
  L3       per-tile h3 lhsT -> w [128e, 4C] PSUM, ACT evac
  products 9 DVE slab mults m (.) w per chunk (+1 ACT copy)
  scatter  one-hot matmuls (Y1 rides in the one-hot weights), pipelined one
           chunk behind products; agg PSUM banks A=[s0|v1x|v1y|v1z]
           B=[v0x|s1x|s1y|v0y] C=[s1z|v0z]
  final    s1 presummed on DVE, 8 transposes, linear_2 + species skip with
           384-col folded matmuls, silu gates; emitted inside the next window
"""
import math
import os
import numpy as np

N, E, C, NS, RB, H = 4000, 128000, 128, 5, 8, 64
AVG_N = 32.0
NCORES = 8
WIN = 128
NWIN = 4
NBIN = NCORES * NWIN
SQ3 = math.sqrt(3.0)
CH = 8                                   # edge tiles per compute chunk


def _prep_host(inputs):
    import ml_dtypes
    bf = ml_dtypes.bfloat16
    f32 = np.float32

    xs = np.asarray(inputs["x_scalars"], f32)
    xv = np.asarray(inputs["x_vectors"], f32)
    ev = np.asarray(inputs["edge_vectors"], f32)
    rb = np.asarray(inputs["radial_basis"], f32)
    W1_0 = np.asarray(inputs["W1_0"], f32)
    W1_1 = np.asarray(inputs["W1_1"], f32)
    w0 = np.asarray(inputs["rmlp_w0"], f32)
    w1 = np.asarray(inputs["rmlp_w1"], f32)
    w2 = np.asarray(inputs["rmlp_w2"], f32)
    w3 = np.asarray(inputs["rmlp_w3"], f32).copy()
    W2_0 = np.asarray(inputs["W2_0"], f32)
    W2_1 = np.asarray(inputs["W2_1"], f32)
    Wsk0 = np.asarray(inputs["Wsk0"], f32)
    Wsk1 = np.asarray(inputs["Wsk1"], f32)
    species = np.asarray(inputs["species"]).astype(np.int64)
    send = np.asarray(inputs["senders"]).astype(np.int64)
    recv = np.asarray(inputs["receivers"]).astype(np.int64)

    inv_c = f32(1.0 / math.sqrt(C))
    W1_0f = W1_0 * inv_c
    W1_1f = W1_1 * inv_c
    w3f = w3 * f32(1.0 / math.sqrt(AVG_N))
    w3f[:, C:2 * C] *= f32(1.0 / SQ3)
    inv_2c = f32(1.0 / math.sqrt(2 * C))
    W2_0f = W2_0 * inv_2c
    W2_1f = W2_1 * inv_2c
    Wsk0f = Wsk0 * inv_c          # [NS, C, 2C]
    Wsk1f = Wsk1 * inv_c          # [NS, C, C]

    # edge geometry (host): Y1 = sqrt(3) * unit(edge_vectors)
    r = np.sqrt((ev * ev).sum(1, keepdims=True))
    Y1 = SQ3 * ev / np.maximum(r, 1e-12)                               # [E,3]

    # ---- balanced node->bin packing (LPT on degree, cap 128 nodes/bin)
    deg = np.bincount(recv, minlength=N)
    order = np.argsort(-deg, kind="stable")
    bin_edges = np.zeros(NBIN, np.int64)
    bin_count = np.zeros(NBIN, np.int64)
    bin_nodes = [[] for _ in range(NBIN)]
    for n in order:
        cand = np.nonzero(bin_count < WIN)[0]
        b = cand[np.argmin(bin_edges[cand])]
        bin_edges[b] += deg[n]
        bin_count[b] += 1
        bin_nodes[b].append(n)
    node_bin = np.zeros(N, np.int64)
    node_slot = np.zeros(N, np.int64)
    for b in range(NBIN):
        for j, n in enumerate(bin_nodes[b]):
            node_bin[n] = b
            node_slot[n] = j

    EW = int(((bin_edges.max() + 255) // 256) * 256)
    TW = EW // 128
    EPAD = EW * NWIN
    TT = EPAD // 128

    ebin = node_bin[recv]
    rloc_all = node_slot[recv]

    # ---- shared constants
    w0b = np.zeros((16, 128), f32)
    w0b[:RB, :H] = w0
    w0b[RB:2 * RB, H:2 * H] = w0
    w1b = np.zeros((128, 128), f32)
    w1b[:H, :H] = w1
    w1b[H:, H:] = w1
    w2b = np.zeros((128, 128), f32)
    w2b[:H, :H] = w2
    w2b[H:, H:] = w2
    w3d = np.concatenate([w3f, w3f], axis=0)                           # [128,4C]

    W20L = np.stack([W2_0f[0:128, :], W2_0f[128:256, :]], axis=1)      # [128,2,256]
    W21L = np.stack([W2_1f[0:128, :], W2_1f[128:256, :]], axis=1)      # [128,2,128]
    Wsk0L = Wsk0f.transpose(1, 0, 2)                                   # [128,NS,256]
    Wsk1L = Wsk1f.transpose(1, 0, 2)                                   # [128,NS,128]

    NPAD = 4096
    xall = np.zeros((C, 4, NPAD), f32)
    xall[:, 0, :N] = xs.T
    for i in range(3):
        xall[:, 1 + i, :N] = xv[:, :, i].T

    consts = dict(
        xall=xall.astype(bf),
        W10=W1_0f.astype(bf), W11=W1_1f.astype(bf),
        w0b=w0b.astype(bf), w1b=w1b.astype(bf), w2b=w2b.astype(bf),
        w3d=w3d.astype(bf),
        W20=W20L.astype(bf), W21=W21L.astype(bf),
        Wsk0=Wsk0L.astype(bf), Wsk1=Wsk1L.astype(bf),
    )

    # ---- per-core tensors
    cores = []
    node_order = np.full((NCORES, NWIN, WIN), -1, np.int64)
    for i in range(NCORES):
        send_p = np.zeros(EPAD, np.int64)
        rloc_p = np.zeros(EPAD, np.int64)
        val_p = np.zeros(EPAD, f32)
        rb_p = np.zeros((EPAD, RB), f32)
        Y1_p = np.zeros((EPAD, 3), f32)
        for w in range(NWIN):
            b = i * NWIN + w
            pw = np.nonzero(ebin == b)[0]
            pw = pw[np.argsort(send[pw], kind="stable")]
            k = len(pw)
            sl = slice(w * EW, w * EW + k)
            send_p[sl] = send[pw]
            rloc_p[sl] = rloc_all[pw]
            val_p[sl] = 1.0
            rb_p[sl] = rb[pw]
            Y1_p[sl] = Y1[pw]
            node_order[i, w, :len(bin_nodes[b])] = bin_nodes[b]

        # gather indices, int16 (pad slots gather row 0; one-hot row is zero)
        # chunk k of each window only gathers y rows < 1024*(k+2) (senders
        # sorted per window; asserted here, baked into the program's deps)
        for w in range(NWIN):
            for kc in range(2):
                seg = send_p[w * EW + kc * 1024: w * EW + (kc + 1) * 1024]
                assert seg.max(initial=0) < 1024 * (kc + 2), "sender skew"
        idx16 = send_p.astype(np.int16).reshape(EPAD // 16, 16).T
        sendidx = np.tile(idx16, (8, 1))                               # [128, EPAD//16]

        # OHW[p, t, 0, n] = onehot; [p, t, 1+i, n] = onehot * Y1_i
        ohw = np.zeros((EPAD, 4, WIN), f32)
        ar = np.arange(EPAD)
        ohw[ar, 0, rloc_p] = val_p
        for j in range(3):
            ohw[ar, 1 + j, rloc_p] = val_p * Y1_p[:, j]
        OHW = ohw.reshape(TT, 128, 4, WIN).transpose(1, 0, 2, 3)

        # rb2[s*8+r, w*(EW/2) + P*128 + p] = rb[edge (w, (2P+s)*128+p), r]
        arr = rb_p.reshape(NWIN, TW // 2, 2, 128, RB)
        rb2 = arr.transpose(2, 4, 0, 1, 3).reshape(16, EPAD // 2)

        xs_my = np.zeros((NWIN * WIN, C), f32)
        xv_my = np.zeros((NWIN * WIN, C, 3), f32)
        soh = np.zeros((NWIN * WIN, NS), f32)
        for w in range(NWIN):
            b = i * NWIN + w
            nb = bin_nodes[b]
            xs_my[w * WIN:w * WIN + len(nb)] = xs[nb]
            xv_my[w * WIN:w * WIN + len(nb)] = xv[nb]
            soh[np.arange(w * WIN, w * WIN + len(nb)), species[nb]] = 1.0
        xskT = np.einsum("nc,nk->ckn", xs_my, soh)                     # [128,NS,512]
        # [C, NS, NWIN, 3, WIN] so the (k, w) slice is 384 contiguous cols
        xvkT = np.einsum("nci,nk->ckin", xv_my, soh).reshape(
            C, NS, 3, NWIN, WIN).transpose(0, 1, 3, 2, 4).reshape(
            C, NS, NWIN, 3 * WIN)

        cores.append(dict(
            sendidx=np.ascontiguousarray(sendidx),
            OHW=np.ascontiguousarray(OHW.astype(bf)),
            rb2=np.ascontiguousarray(rb2.astype(bf)),
            xskT=np.ascontiguousarray(xskT.astype(bf)),
            xvkT=np.ascontiguousarray(xvkT.astype(bf)),
        ))
    return consts, cores, EW, node_order


def _build_program(EW):
    import concourse.bacc as bacc
    import concourse.mybir as mybir
    import concourse.tile as tile
    import concourse.hw_specs as hw_specs
    from concourse.masks import make_identity

    # calibrate the scheduler's SWDGE descriptor-generation cost to measured
    # hardware (~8.4 ns/descriptor) so gather latency is modeled realistically
    _swdge_saved = hw_specs.TRN2Spec.SWDGE_NS_PER_DESCRIPTOR
    hw_specs.TRN2Spec.SWDGE_NS_PER_DESCRIPTOR = 8.4

    f32 = mybir.dt.float32
    bf = mybir.dt.bfloat16
    AF = mybir.ActivationFunctionType
    OP = mybir.AluOpType

    TW = EW // 128
    EPAD = EW * NWIN
    TT = EPAD // 128

    nc = bacc.Bacc("TRN2", target_bir_lowering=False)

    def param(name, shape, dtype):
        return nc.declare_dram_parameter(name, list(shape), dtype, isOutput=False)

    NPAD = 4096
    i16 = mybir.dt.int16
    xall_d = param("xall", (C, 4, NPAD), bf)
    sendidx_d = param("sendidx", (128, EPAD // 16), i16)
    W10_d = param("W10", (C, C), bf)
    W11_d = param("W11", (C, C), bf)
    w0b_d = param("w0b", (16, 128), bf)
    w1b_d = param("w1b", (128, 128), bf)
    w2b_d = param("w2b", (128, 128), bf)
    w3d_d = param("w3d", (128, 4 * C), bf)
    W20_d = param("W20", (C, 2, 2 * C), bf)
    W21_d = param("W21", (C, 2, C), bf)
    Wsk0_d = param("Wsk0", (C, NS, 2 * C), bf)
    Wsk1_d = param("Wsk1", (C, NS, C), bf)
    OHW_d = param("OHW", (128, TT, 4, WIN), bf)
    rb2_d = param("rb2", (16, EPAD // 2), bf)
    xskT_d = param("xskT", (C, NS, NWIN * WIN), bf)
    xvkT_d = param("xvkT", (C, NS, NWIN, 3 * WIN), bf)
    outT_d = nc.declare_dram_parameter("outT", [4 * C, NWIN * WIN], f32,
                                       isOutput=True)

    with tile.TileContext(nc) as tc:
        with (
            tc.tile_pool(name="dram", bufs=1, space="DRAM") as dpool,
            tc.tile_pool(name="const", bufs=1) as cpool,
            tc.tile_pool(name="xload", bufs=2) as xpool,
            tc.tile_pool(name="rbload", bufs=2) as rbpool,
            tc.tile_pool(name="hslab", bufs=2) as hpool,
            tc.tile_pool(name="eload", bufs=2) as epool,
            tc.tile_pool(name="mw", bufs=3) as mwpool,
            tc.tile_pool(name="prod", bufs=2) as ppool,
            tc.tile_pool(name="fin", bufs=2) as fpool,
            tc.tile_pool(name="ps_w", bufs=4, space="PSUM") as ps_w,
            tc.tile_pool(name="ps_f", bufs=1, space="PSUM") as ps_f,
            tc.tile_pool(name="ps_agg", bufs=1, space="PSUM") as ps_agg,
        ):
            y_d = dpool.tile([NPAD, 4 * C], bf)

            def cload(dram, shape, dtype):
                t = cpool.tile(list(shape), dtype, tag=dram.name)
                nc.scalar.dma_start(t[:], dram[:])
                return t

            W10_s = cload(W10_d, (C, C), bf)
            W11_s = cload(W11_d, (C, C), bf)
            w0b_s = cload(w0b_d, (16, 128), bf)
            w1b_s = cload(w1b_d, (128, 128), bf)
            w2b_s = cload(w2b_d, (128, 128), bf)
            w3d_s = cload(w3d_d, (128, 4 * C), bf)
            sendidx_s = cload(sendidx_d, (128, EPAD // 16), i16)

            # ====== phase Y: y = x @ W1 (all nodes), quarter-pipelined ======
            # y_d written partition-major ([128, 32, 512]); gather indices are
            # host-remapped to match, so each quarter is a single DMA
            y_v = y_d[:].rearrange("(c p) f -> p c f", p=128)
            aggY = ps_agg.tile([128, 3, 512], f32, tag="agg")
            yq = None
            for g in range(8):
                xe = xpool.tile([C, 4, 512], bf, tag="xq", bufs=3)
                nc.sync.dma_start(xe[:], xall_d[:, :, g * 512:(g + 1) * 512])
                if g % 2 == 0:
                    yq = xpool.tile([128, 8, 4 * C], bf, tag="yq", name="yq")
                for nch in range(4):
                    k = g * 4 + nch
                    csl = slice(nch * 128, (nch + 1) * 128)
                    k7 = k % 7
                    if k7 < 4:
                        psy = ps_w.tile([128, 4 * C], f32, tag="w", name="psy")
                    else:
                        psy = aggY[:, k7 - 4, :]
                    nc.tensor.matmul(psy[:, 0:C], lhsT=xe[:, 0, csl],
                                     rhs=W10_s[:], start=True, stop=True)
                    for i in range(3):
                        nc.tensor.matmul(psy[:, (1 + i) * C:(2 + i) * C],
                                         lhsT=xe[:, 1 + i, csl],
                                         rhs=W11_s[:], start=True, stop=True)
                    if k % 2 == 0:
                        nc.vector.tensor_copy(yq[:, k % 8, :], psy[:])
                    else:
                        nc.scalar.copy(yq[:, k % 8, :], psy[:])
                if g % 2 == 1:
                    q = g // 2
                    nc.scalar.dma_start(y_v[:, q * 8:(q + 1) * 8, :], yq[:])

            ncol0 = EW // 2
            rb2_w0 = rbpool.tile([16, ncol0], bf, tag="rb2")
            nc.scalar.dma_start(rb2_w0[:], rb2_d[:, 0:ncol0])
            xskT_w0 = fpool.tile([C, NS, WIN], bf, tag="xsk")
            nc.scalar.dma_start(xskT_w0[:], xskT_d[:, :, 0:WIN])
            xvkT_w0 = fpool.tile([C, NS, 3 * WIN], bf, tag="xvk")
            nc.scalar.dma_start(xvkT_w0[:], xvkT_d[:, :, 0, :])

            # final-stage constants (not needed until window 1)
            W20_s = cload(W20_d, (C, 2, 2 * C), bf)
            W21_s = cload(W21_d, (C, 2, C), bf)
            Wsk0_s = cload(Wsk0_d, (C, NS, 2 * C), bf)
            Wsk1_s = cload(Wsk1_d, (C, NS, C), bf)
            ident_s = cpool.tile([128, 128], bf)
            make_identity(nc, ident_s[:])

            pending_final = [None]

            for w in range(NWIN):
                # ---------------- radial MLP (2-slot packed) ----------------
                ncol = EW // 2
                if w == 0:
                    rb2_t, xskT_w, xvkT_w = rb2_w0, xskT_w0, xvkT_w0
                else:
                    rb2_t = rbpool.tile([16, ncol], bf, tag="rb2")
                    nc.sync.dma_start(rb2_t[:], rb2_d[:, w * ncol:(w + 1) * ncol])
                    xskT_w = fpool.tile([C, NS, WIN], bf, tag="xsk")
                    nc.sync.dma_start(xskT_w[:],
                                      xskT_d[:, :, w * WIN:(w + 1) * WIN])
                    xvkT_w = fpool.tile([C, NS, 3 * WIN], bf, tag="xvk")
                    nc.sync.dma_start(xvkT_w[:], xvkT_d[:, :, w, :])
                h3 = hpool.tile([128, ncol], bf, tag="h3")
                h1 = hpool.tile([128, 2, 512], bf, tag="h1")
                h2 = hpool.tile([128, 2, 512], bf, tag="h2")

                def radial_pair(cA, cB):
                    # two 512-col chunks interleaved so PE never waits on silu
                    cols = [(cA, 0), (cB, 1)]
                    W = [w0b_s, w1b_s, w2b_s]
                    src_h = [None, h1, h2]
                    ps = {}
                    for l in range(3):
                        for (c0, p) in cols:
                            pst = ps_w.tile([128, 512], f32, tag="w",
                                            name=f"psr{l}{p}")
                            if l == 0:
                                nc.tensor.matmul(pst[:], lhsT=W[0][:],
                                                 rhs=rb2_t[:, c0:c0 + 512],
                                                 start=True, stop=True)
                            else:
                                nc.tensor.matmul(pst[:], lhsT=W[l][:],
                                                 rhs=src_h[l][:, p, :],
                                                 start=True, stop=True)
                            ps[p] = pst
                        for (c0, p) in cols:
                            if l < 2:
                                nc.scalar.activation(src_h[l + 1][:, p, :],
                                                     ps[p][:], AF.Silu)
                            else:
                                nc.scalar.activation(h3[:, c0:c0 + 512],
                                                     ps[p][:], AF.Silu)

                # banks: A=[s0|v1x|v1y|v1z]  B=[v0x|s1x|s1y|v0y]  Cb=[s1z|v0z]
                agg = ps_agg.tile([128, 3, 512], f32, tag="agg")

                def emit_scatter(st):
                    (s_t0, s_t1, s_ohw, s_P1, s_P2, s_P3) = st
                    for tl in range(s_t1 - s_t0):
                        t = s_t0 + tl
                        first = (t == 0)
                        last = (t == TW - 1)
                        oh = s_ohw[:, tl, 0, :]
                        ohx = s_ohw[:, tl, 1, :]
                        ohy = s_ohw[:, tl, 2, :]
                        ohz = s_ohw[:, tl, 3, :]
                        nc.tensor.matmul(agg[:, 0, :], lhsT=oh,
                                         rhs=s_P1[:, tl, 0:512],
                                         start=first, stop=last,
                                         skip_group_check=True)
                        nc.tensor.matmul(agg[:, 1, 0:256], lhsT=ohx,
                                         rhs=s_P2[:, tl, 128:384],
                                         start=first, stop=False,
                                         skip_group_check=True)
                        nc.tensor.matmul(agg[:, 1, 256:512], lhsT=ohy,
                                         rhs=s_P2[:, tl, 0:256],
                                         start=False, stop=last,
                                         skip_group_check=True)
                        nc.tensor.matmul(agg[:, 2, 0:256], lhsT=ohz,
                                         rhs=s_P3[:, tl, 0:256],
                                         start=first, stop=last,
                                         skip_group_check=True)

                pending = None
                chunks = [(t0, min(t0 + CH, TW)) for t0 in range(0, TW, CH)]
                for (t0, t1) in chunks:
                    nt = t1 - t0
                    g0 = w * TW + t0
                    # dense scatter block first (keeps the PE p-state ramped)
                    if pending is not None:
                        emit_scatter(pending)
                        pending = None
                    # radial cols [t0*64, t1*64) feed edge tiles [t0, t1);
                    # pairs emitted at chunks 0 and 2
                    if t0 % (2 * CH) == 0:
                        radial_pair(t0 * 64, (t0 + CH) * 64)
                    if t0 == 0 and pending_final[0] is not None:
                        pending_final[0]()
                        pending_final[0] = None
                    # ---- loads for this chunk
                    ohw_t = epool.tile([128, CH, 4, WIN], bf, tag="ohw", bufs=2)
                    nc.sync.dma_start(ohw_t[:, :nt], OHW_d[:, g0:g0 + nt, :, :])

                    # ---- m = y[send] via SWDGE gather
                    m_sb = mwpool.tile([128, CH, 512], bf, tag="m", bufs=4)
                    w_sb = mwpool.tile([128, CH, 512], bf, tag="w", bufs=3)
                    nidx = nt * 128
                    ylim = min(1024 * (t0 // CH + 2), NPAD)
                    nc.gpsimd.dma_gather(
                        m_sb[:, 0:nt, :], y_d[0:ylim, :],
                        sendidx_s[:, g0 * 8:g0 * 8 + nidx // 16],
                        nidx, nidx, 4 * C,
                    )
                    for tl in range(nt):
                        t = t0 + tl
                        s = t % 2
                        P = t // 2
                        psw = ps_w.tile([128, 512], f32, tag="w", name="psw")
                        nc.tensor.matmul(
                            psw[:], lhsT=h3[s * 64:(s + 1) * 64,
                                            P * 128:(P + 1) * 128],
                            rhs=w3d_s[s * 64:(s + 1) * 64, :],
                            start=True, stop=True)
                        if tl % 2 == 0:
                            nc.scalar.copy(w_sb[:, tl, :], psw[:])
                        else:
                            nc.vector.tensor_copy(w_sb[:, tl, :], psw[:])

                    # ---- products (DVE slabs over the chunk)
                    # w_sb cols: [ws0 | ws1' | wv0 | wv1]; m_sb: [m0|m1x|m1y|m1z]
                    P1 = ppool.tile([128, CH, 512], bf, tag="P1", bufs=2)
                    P2 = ppool.tile([128, CH, 384], bf, tag="P2", bufs=2)
                    P3 = ppool.tile([128, CH, 256], bf, tag="P3", bufs=2)

                    def mslice(j):
                        return m_sb[:, 0:nt, j * 128:(j + 1) * 128]

                    def wslice(j):
                        return w_sb[:, 0:nt, j * 128:(j + 1) * 128]

                    # P1 = [m0*ws0 | m1x*wv1 | m1y*wv1 | m1z*wv1]
                    nc.vector.tensor_tensor(out=P1[:, 0:nt, 0:128],
                                            in0=mslice(0), in1=wslice(0), op=OP.mult)
                    for j in range(3):
                        nc.vector.tensor_tensor(
                            out=P1[:, 0:nt, (1 + j) * 128:(2 + j) * 128],
                            in0=mslice(1 + j), in1=wslice(3), op=OP.mult)
                    # P2 = [m1y*ws1' | m0*wv0 | m1x*ws1']
                    nc.vector.tensor_tensor(out=P2[:, 0:nt, 0:128],
                                            in0=mslice(2), in1=wslice(1), op=OP.mult)
                    nc.vector.tensor_tensor(out=P2[:, 0:nt, 128:256],
                                            in0=mslice(0), in1=wslice(2), op=OP.mult)
                    nc.vector.tensor_tensor(out=P2[:, 0:nt, 256:384],
                                            in0=mslice(1), in1=wslice(1), op=OP.mult)
                    # P3 = [m1z*ws1' | m0*wv0 (copy)]
                    nc.vector.tensor_tensor(out=P3[:, 0:nt, 0:128],
                                            in0=mslice(3), in1=wslice(1), op=OP.mult)
                    nc.scalar.copy(P3[:, 0:nt, 128:256], P2[:, 0:nt, 128:256])

                    # ---- scatter pipelined one chunk behind
                    pending = (t0, t1, ohw_t, P1, P2, P3)

                if pending is not None:
                    emit_scatter(pending)
                    pending = None

                # ================= final per window =================
                def make_final(w, agg, xskT_w=xskT_w, xvkT_w=xvkT_w):
                    def emit_final():
                        # agg_sb blocks: 0=s0 1=s1sum 2=v0x 3=v0y 4=v0z
                        #                5=v1x 6=v1y 7=v1z
                        agg_sb = fpool.tile([128, 8, 128], bf, tag="aggsb")
                        nc.scalar.copy(agg_sb[:, 0, :], agg[:, 0, 0:128])
                        nc.scalar.copy(agg_sb[:, 5:8, :], agg[:, 0, 128:512])
                        nc.scalar.copy(agg_sb[:, 2, :], agg[:, 1, 0:128])
                        nc.scalar.copy(agg_sb[:, 3, :], agg[:, 1, 384:512])
                        nc.scalar.copy(agg_sb[:, 4, :], agg[:, 2, 128:256])
                        s1t = fpool.tile([128, 2, 128], f32, tag="s1t")
                        nc.scalar.copy(s1t[:, 0, :], agg[:, 1, 128:256])
                        nc.vector.tensor_tensor(out=s1t[:, 1, :],
                                                in0=agg[:, 1, 256:384],
                                                in1=s1t[:, 0, :], op=OP.add)
                        nc.vector.tensor_tensor(out=agg_sb[:, 1, :],
                                                in0=agg[:, 2, 0:128],
                                                in1=s1t[:, 1, :], op=OP.add)

                        aggT = fpool.tile([128, 8, 128], bf, tag="aggT")
                        for b in range(8):
                            pst = ps_f.tile([128, 128], bf, tag="f", name="pst")
                            nc.tensor.transpose(pst[:], agg_sb[:, b, :],
                                                identity=ident_s[:])
                            if b % 2 == 0:
                                nc.vector.tensor_copy(aggT[:, b, :], pst[:])
                            else:
                                nc.scalar.copy(aggT[:, b, :], pst[:])

                        pss = ps_f.tile([128, 2, 128], f32, tag="f", name="pss")
                        for mch in range(2):
                            nc.tensor.matmul(
                                pss[:, mch, :],
                                lhsT=W20_s[:, 0, mch * 128:(mch + 1) * 128],
                                rhs=aggT[:, 0, :], start=True, stop=False,
                                skip_group_check=True)
                            nc.tensor.matmul(
                                pss[:, mch, :],
                                lhsT=W20_s[:, 1, mch * 128:(mch + 1) * 128],
                                rhs=aggT[:, 1, :], start=False, stop=False,
                                skip_group_check=True)
                            for k in range(NS):
                                nc.tensor.matmul(
                                    pss[:, mch, :],
                                    lhsT=Wsk0_s[:, k, mch * 128:(mch + 1) * 128],
                                    rhs=xskT_w[:, k, :],
                                    start=False, stop=(k == NS - 1),
                                    skip_group_check=True)
                        outs = fpool.tile([128, 128], bf, tag="outs")
                        nc.scalar.activation(outs[:], pss[:, 0, :], AF.Silu)
                        gates = fpool.tile([128, 128], bf, tag="gates")
                        nc.scalar.activation(gates[:], pss[:, 1, :], AF.Silu)

                        psv = ps_f.tile([128, 3, 128], f32, tag="f", name="psv")
                        nc.tensor.matmul(psv[:, :, :], lhsT=W21_s[:, 0, :],
                                         rhs=aggT[:, 2:5, :],
                                         start=True, stop=False,
                                         skip_group_check=True)
                        nc.tensor.matmul(psv[:, :, :], lhsT=W21_s[:, 1, :],
                                         rhs=aggT[:, 5:8, :],
                                         start=False, stop=False,
                                         skip_group_check=True)
                        for k in range(NS):
                            nc.tensor.matmul(
                                psv[:, :, :], lhsT=Wsk1_s[:, k, :],
                                rhs=xvkT_w[:, k, :],
                                start=False, stop=(k == NS - 1),
                                skip_group_check=True)

                        ow = fpool.tile([128, 4, 128], f32, tag="ow")
                        nc.vector.tensor_copy(ow[:, 0, :], outs[:])
                        for i in range(3):
                            nc.vector.tensor_tensor(out=ow[:, 1 + i, :],
                                                    in0=psv[:, i, :],
                                                    in1=gates[:], op=OP.mult)
                        for fch in range(4):
                            nc.sync.dma_start(
                                outT_d[fch * 128:(fch + 1) * 128,
                                       w * 128:(w + 1) * 128],
                                ow[:, fch, :])
                    return emit_final

                pending_final[0] = make_final(w, agg)

            if pending_final[0] is not None:
                pending_final[0]()
                pending_final[0] = None
    nc.compile()
    hw_specs.TRN2Spec.SWDGE_NS_PER_DESCRIPTOR = _swdge_saved
    return nc


_CACHE = {}


def kernel(**inputs):
    from concourse.bass_utils import run_bass_kernel_spmd
    consts, cores, EW, node_order = _prep_host(inputs)
    if EW not in _CACHE:
        _CACHE[EW] = _build_program(EW)
    nc = _CACHE[EW]
    in_maps = []
    for i in range(NCORES):
        m = dict(consts)
        m.update(cores[i])
        in_maps.append(m)
    res = run_bass_kernel_spmd(nc, in_maps, list(range(NCORES)))
    out = np.zeros((N, 4 * C), np.float32)
    for i in range(NCORES):
        outT = np.asarray(res.results[i]["outT"], np.float32)   # [512, 512]
        full = outT.T                                           # [512, 512]
        for w in range(NWIN):
            nodes = node_order[i, w]
            valid = nodes >= 0
            rows = full[w * WIN:(w + 1) * WIN][valid]
            out_s = rows[:, 0:C]
            v = np.stack([rows[:, C:2 * C], rows[:, 2 * C:3 * C],
                          rows[:, 3 * C:]], axis=2).reshape(-1, 3 * C)
            out[nodes[valid]] = np.concatenate([out_s, v], axis=1)
    return out
